# revision 43
# baseline (speedup 1.0000x reference)
"""MoE top-1 feed-forward (DeepSpeed-style) on 8 Trainium2 NeuronCores.

Strategy (expert parallelism, per the sharding hint):
  - Host computes the (tiny) gate: logits = x @ Wg, softmax, top-1 expert id
    and gate prob per token (float64 for a faithful argmax).
  - Tokens are dispatched to the core owning their expert (core e holds
    W1[e]/b1[e]/W2[e]/b2[e]); each core's token batch is padded to a common
    capacity C so all 8 cores run one SPMD program.
  - Each core runs the dense FFN for its tokens:
        hT = silu(W1^T @ xT + b1);  yT = W2^T @ hT
    with tokens laid out along the free (moving) dimension so no transposes
    are needed on device: xT is [D, C], hT is [F, C], yT is [D, C].
  - All weights are SBUF-resident (~75KB/partition in bf16) and their DMAs
    are issued eagerly up-front, striped over the three DMA-capable rings
    (SP / ACT / Pool) in consumption order. Measured on HW: no ring moves
    data until ~9-10us after kernel start and each queue tops out around
    80-135 GB/s, so all three must stream continuously; the PE start is
    gated on sentinel tiles so that once running it never stalls (a stall
    also costs a ~1.5us p-state re-ramp).
  - mm2 trails mm1 by four chunks (PSUM-resident y accumulators), so late
    W2 arrivals don't bubble the PE; the tail evacuates PSUM on DVE+ACT
    casting to bf16 and stores via two DMAs on the idle SP/ACT rings.
  - Host combines: out[token] = gate * (y + b2[expert]).
"""

import os
import sys

import numpy as np

try:
    import concourse.mybir as mybir  # noqa: F401
except ModuleNotFoundError:  # fallback if the site hooks aren't installed
    sys.path.insert(0, "/opt/trn_rl_repo")

import concourse.mybir as mybir
import concourse.tile as tile
from concourse import bacc
from concourse.bass_utils import run_bass_kernel_spmd

N_CORES = 8

# Compute dtype for the matmuls:
#   "bf16" - weights/activations cast to bfloat16 (f32 PSUM accumulate).
#            Same 1 cycle/row PE rate as f32r but half the HBM traffic,
#            which is what this kernel is limited by.
#   "f32r" - fp32 data, PE's replicated-fp32 mode (full rate at N>=256)
#   "f32"  - plain fp32 matmuls (4x slower PE)
MODE = os.environ.get("BASS_MOE_MODE", "bf16")

FG = int(os.environ.get("BASS_MOE_FG", "2"))  # steady-state f-chunks per W1 group
W2P = int(os.environ.get("BASS_MOE_W2P", "2"))  # f-chunks per W2 pair-tile


def _w1_groups(KF):
    """F-chunk widths per W1 group: small leading groups let the PE start
    before a whole FG-wide image lands, and small steady-state groups keep
    each DMA under ~400KB so the ~120GB/s-per-queue rings interleave finely."""
    lead = [1, 1] if KF > 8 and FG >= 2 else ([2, 2] if FG > 2 and KF > 4 else [])
    rem = KF - sum(lead)
    groups = list(lead)
    while rem > 0:
        w = min(FG, rem)
        groups.append(w)
        rem -= w
    return groups


_CACHE: dict = {}


def _roundup(a: int, m: int) -> int:
    return -(-a // m) * m


def _build_bass(C: int, n_slabs: int, mode: str, D: int, F: int):
    """Build + compile the per-core Bass program for capacity C (divisible by
    n_slabs; slab width CS = C/n_slabs must be 256..512)."""
    f32 = mybir.dt.float32
    if mode == "bf16":
        dt_io = mybir.dt.bfloat16
    elif mode == "f32r":
        dt_io = mybir.dt.float32r
    else:
        dt_io = f32

    KD, KF = D // 128, F // 128
    GRPS = _w1_groups(KF)
    NP = KF // W2P  # number of W2 pair-tiles
    CS = C // n_slabs
    assert C % n_slabs == 0 and 256 <= CS <= 512

    nc = bacc.Bacc(None, target_bir_lowering=False, debug=False)
    # Host-packed images (see kernel() for the packing):
    #   xT   [128, KD*C]             col d*C+t = x^T[d*128+p, t]
    #   w1   [128, KD*F]             flat group images; group g at column
    #                                offset KD*128*sum(GRPS[:g]), blocks (d, j)
    #                                within a group at (d*gw+j)*128
    #   w2   [NP, 128, W2P*D]        w2[p] f-chunk r=f-p*W2P at cols r*D
    #   b1r  [128, KF]               b1[f*128+p] at [p, f]
    #   yT   [128, KD*C]             output, same layout as xT (dt_io)
    HX = KD // 2
    xA = nc.dram_tensor("xA", [128, HX * C], dt_io, kind="ExternalInput")
    xB = nc.dram_tensor("xB", [128, (KD - HX) * C], dt_io, kind="ExternalInput")
    w1 = nc.dram_tensor("w1", [128 * KD * F], dt_io, kind="ExternalInput")
    w2 = nc.dram_tensor("w2", [NP, 128, W2P * D], dt_io, kind="ExternalInput")
    b1r = nc.dram_tensor("b1r", [128, KF], f32, kind="ExternalInput")
    yA = nc.dram_tensor("yA", [128, HX * C], dt_io, kind="ExternalOutput")
    yB = nc.dram_tensor("yB", [128, (KD - HX) * C], dt_io, kind="ExternalOutput")

    silu = mybir.ActivationFunctionType.Silu

    with tile.TileContext(nc) as tc:
        with (
            tc.tile_pool(name="xp", bufs=1) as xp,
            tc.tile_pool(name="wp", bufs=1) as wp,
            tc.tile_pool(name="hp", bufs=6) as hp,
            tc.tile_pool(name="bp", bufs=1) as bp,
            tc.tile_pool(name="yp", bufs=2) as yp,
            tc.tile_pool(name="ps_h", bufs=2, space="PSUM") as ps_h,
            tc.tile_pool(name="ps_y", bufs=1, space="PSUM") as ps_y,
        ):
            # ---- tiles ----
            b1t = bp.tile([128, KF], f32, tag="b1", name="b1t")
            # x arrives as two half-width images on different rings so the
            # first mm1 isn't gated on one queue moving the whole 0.4MB
            xwa = [
                xp.tile([128, HX * CS], dt_io, tag=f"xwa{s}", name=f"xwa{s}")
                for s in range(n_slabs)
            ]
            xwb = [
                xp.tile([128, (KD - HX) * CS], dt_io, tag=f"xwb{s}", name=f"xwb{s}")
                for s in range(n_slabs)
            ]
            w1ts = []
            f0 = 0
            w1_offs = []
            for g, gw in enumerate(GRPS):
                w1ts.append(
                    wp.tile([128, KD * gw * 128], dt_io, tag=f"w1_{g}", name=f"w1t{g}")
                )
                w1_offs.append(f0)
                f0 += gw
            w2ts = [
                wp.tile([128, W2P * D], dt_io, tag=f"w2_{p}", name=f"w2t{p}")
                for p in range(NP)
            ]

            def load_w1(eng, g):
                # each group is a contiguous partition-major DRAM block so
                # the SDMA merges partition lines into large packets (the
                # queues are packet-rate bound)
                o = 128 * KD * 128 * w1_offs[g]
                w = KD * GRPS[g] * 128
                eng.dma_start(
                    out=w1ts[g][:],
                    in_=w1[o : o + 128 * w].rearrange("(p w) -> p w", p=128),
                )

            def load_w2(eng, p):
                eng.dma_start(out=w2ts[p][:], in_=w2[p])

            # ---- load scheduling ----
            # Only SP / ACT / Pool can initiate DMAs; no ring moves data until
            # ~9-10us after kernel start (DGE spin-up) and EACH queue tops out
            # around 110-130 GB/s regardless of HWDGE/SWDGE, so all three
            # queues must stream continuously. Items are striped across the
            # rings with a greedy earliest-completion schedule (computed
            # against the measured queue starts/rates) so every tile lands a
            # few us before the PE consumes it.
            ng = len(GRPS)
            if ng == 13 and NP == 12:
                # Stripe at measured queue rates (sync ~100, act ~80, pool
                # ~135 GB/s; first data ~8.5/9/9.5us). The PE start is gated
                # on g2's arrival (~14.4us) so that from there on every tile
                # lands before the PE needs it and the PE runs one unbroken
                # stretch - each stall also costs a ~1.5us p-state re-ramp,
                # so stalls are doubly expensive.
                sync_items = [("g", 1), ("g", 3), ("g", 5), ("p", 5), ("p", 7),
                              ("g", 9), ("p", 8), ("p", 11)]
                act_pre = [("p", 0), ("p", 2), ("p", 3)]
                act_mid = [(4, "p", 6), (8, "p", 9)]
                pool_items = [("g", 0), ("g", 2), ("g", 4), ("p", 1), ("g", 6),
                              ("g", 7), ("p", 4), ("g", 8), ("g", 10), ("g", 11),
                              ("g", 12), ("p", 10)]
                gate_tiles = (2, 3)  # w1 groups whose arrival releases the x fanout
            else:  # generic fallback (not tuned)
                sync_items = [("g", g) for g in range(ng)]
                act_pre = [("p", p) for p in range(min(2, NP))]
                act_mid = []
                pool_items = [("p", p) for p in range(min(2, NP), NP)]
                gate_tiles = ()

            def load(eng, kind, i):
                (load_w1 if kind == "g" else load_w2)(eng, i)

            xAv = xA.rearrange("p (k c) -> p k c", k=HX)
            xBv = xB.rearrange("p (k c) -> p k c", k=KD - HX)

            def load_x(s):
                # the halves are contiguous DRAM blocks: one wide packet run
                if n_slabs == 1:
                    nc.sync.dma_start(out=xwa[0][:], in_=xA[:])
                    nc.scalar.dma_start(out=xwb[0][:], in_=xB[:])
                else:
                    nc.sync.dma_start(
                        out=xwa[s][:], in_=xAv[:, :, s * CS : (s + 1) * CS]
                    )
                    nc.scalar.dma_start(
                        out=xwb[s][:], in_=xBv[:, :, s * CS : (s + 1) * CS]
                    )

            # sync: first w1 group leads, then xwa - the x arrival (plus the
            # DVE fanout) is what releases the first mm1, so its ring
            # position paces the PE start without any reorderable gate op
            nc.scalar.dma_start(out=b1t[:], in_=b1r[:])
            load_x(0)
            for kind, i in sync_items:
                load(nc.sync, kind, i)
            for kind, i in act_pre:
                load(nc.scalar, kind, i)
            for kind, i in pool_items:
                load(nc.gpsimd, kind, i)
            for s in range(1, n_slabs):
                load_x(s)

            for s in range(n_slabs):
                c0 = s * CS
                # fan the wide x images out to narrow per-d tiles on the
                # (otherwise idle) vector engine; narrow rhs tiles keep the
                # PE moving-operand read on its fast path
                xt = []
                for d in range(KD):
                    src = xwa[s] if d < HX else xwb[s]
                    off = d if d < HX else d - HX
                    t = xp.tile([128, CS], dt_io, tag=f"x{d}", name=f"xt{d}")
                    nc.vector.tensor_copy(t[:], src[:, off * CS : (off + 1) * CS])
                    xt.append(t)

                def xsl(d):
                    return xt[d][:]

                py = [
                    ps_y.tile([128, CS], f32, tag=f"y{dd}", name=f"py{dd}")
                    for dd in range(KD)
                ]

                def emit_mm2(f, ht):
                    # yT += W2[f-chunk, :]^T @ hT[f-chunk]
                    p, r = divmod(f, W2P)
                    for dd in range(KD):
                        nc.tensor.matmul(
                            py[dd][:],
                            w2ts[p][:, r * D + dd * 128 : r * D + (dd + 1) * 128],
                            ht[:],
                            start=(f == 0),
                            stop=(f == KF - 1),
                        )

                # mm2 is deferred four chunks behind mm1: chunk f's silu runs
                # while mm1(f+1) is on the PE, and the extra slots ride out
                # late W2 arrivals during the DMA-ring start seam.
                pend: list = []
                f0 = 0
                for g, gw in enumerate(GRPS):
                    for j in range(gw):
                        f = f0 + j
                        # hT[f-chunk] = silu(sum_d W1[d, f-chunk]^T @ xT[d] + b1)
                        ph = ps_h.tile([128, CS], f32, tag="hps", name="ph")
                        for d in range(KD):
                            nc.tensor.matmul(
                                ph[:],
                                w1ts[g][:, (d * gw + j) * 128 : (d * gw + j + 1) * 128],
                                xsl(d),
                                start=(d == 0),
                                stop=(d == KD - 1),
                            )
                        ht = hp.tile([128, CS], dt_io, tag="ht", name="ht")
                        nc.scalar.activation(ht[:], ph[:], silu, bias=b1t[:, f : f + 1])
                        if s == 0 and act_mid and act_mid[0][0] == f:
                            _, kind, i = act_mid.pop(0)
                            load(nc.scalar, kind, i)
                        pend.append((f, ht))
                        if len(pend) > 4:
                            emit_mm2(*pend.pop(0))
                    f0 += gw
                while pend:
                    emit_mm2(*pend.pop(0))

                # tail: evacuate PSUM on both DVE and ACT (casting to dt_io),
                # stream out in two DMAs on the SP and ACT rings
                yt = yp.tile([128, KD * CS], dt_io, tag="yt", name="yt")
                half = KD // 2
                for dd in range(KD):
                    if dd < half:
                        nc.vector.tensor_copy(
                            yt[:, dd * CS : (dd + 1) * CS], py[dd][:]
                        )
                    else:
                        nc.scalar.copy(yt[:, dd * CS : (dd + 1) * CS], py[dd][:])
                if n_slabs == 1:
                    # pool ring: SWDGE merges the contiguous store into large
                    # packets (the HWDGE rings emit slow 1632B per-line ones)
                    nc.gpsimd.dma_start(out=yA[:], in_=yt[:, 0 : half * CS])
                    nc.gpsimd.dma_start(out=yB[:], in_=yt[:, half * CS :])
                else:
                    yAv = yA.rearrange("p (k c) -> p k c", k=half)
                    yBv = yB.rearrange("p (k c) -> p k c", k=KD - half)
                    nc.sync.dma_start(
                        out=yAv[:, :, c0 : c0 + CS], in_=yt[:, 0 : half * CS]
                    )
                    nc.scalar.dma_start(
                        out=yBv[:, :, c0 : c0 + CS], in_=yt[:, half * CS :]
                    )

    nc.compile()
    return nc


def _get_bass(C: int, n_slabs: int, mode: str, D: int, F: int):
    key = (C, n_slabs, mode, D, F, FG, W2P)
    if key not in _CACHE:
        _CACHE[key] = _build_bass(C, n_slabs, mode, D, F)
    return _CACHE[key]


def _gate_host(x: np.ndarray, Wg: np.ndarray):
    """Top-1 gating in float64: returns (expert_idx [T], gate [T] f32)."""
    logits = x.astype(np.float64) @ Wg.astype(np.float64)
    m = logits.max(-1, keepdims=True)
    p = np.exp(logits - m)
    p /= p.sum(-1, keepdims=True)
    return p.argmax(-1), p.max(-1).astype(np.float32)


def _kernel_numpy(x, Wg, W1, b1, W2, b2):
    """Reference-equivalent fallback (host only)."""
    idx, gate = _gate_host(x, Wg)
    out = np.zeros_like(x)
    for e in range(W1.shape[0]):
        ids = np.nonzero(idx == e)[0]
        if ids.size == 0:
            continue
        h = x[ids] @ W1[e] + b1[e]
        h = h * (1.0 / (1.0 + np.exp(-h)))
        out[ids] = gate[ids, None] * (h @ W2[e] + b2[e])
    return out


def kernel(hidden_states, Wg, W1, b1, W2, b2):
    hidden_states = np.asarray(hidden_states)
    Wg = np.asarray(Wg, dtype=np.float32)
    W1 = np.asarray(W1, dtype=np.float32)
    b1 = np.asarray(b1, dtype=np.float32)
    W2 = np.asarray(W2, dtype=np.float32)
    b2 = np.asarray(b2, dtype=np.float32)

    orig_shape = hidden_states.shape
    D = orig_shape[-1]
    x = np.ascontiguousarray(hidden_states, dtype=np.float32).reshape(-1, D)
    E, _, F = W1.shape
    KD, KF = D // 128, F // 128

    if E != N_CORES or D % 128 != 0 or F % 128 != 0 or KF % FG != 0:
        return _kernel_numpy(x, Wg, W1, b1, W2, b2).reshape(orig_shape)

    idx, gate = _gate_host(x, Wg)
    order = np.argsort(idx, kind="stable")
    counts = np.bincount(idx, minlength=E)
    starts = np.concatenate([[0], np.cumsum(counts)])

    # Capacity: common padded token count per core. Slab width must be
    # 256..512 (PSUM bank limit / fp32r fast path).
    C = max(256, _roundup(int(counts.max()), 16))
    n_slabs = -(-C // 512)
    C = n_slabs * max(256, _roundup(-(-C // n_slabs), 16))

    mode = MODE
    np_io = np.float32
    if mode == "bf16":
        import ml_dtypes

        np_io = ml_dtypes.bfloat16

    nc = _get_bass(C, n_slabs, mode, D, F)

    NP = KF // W2P
    in_maps = []
    for e in range(E):
        ids = order[starts[e] : starts[e + 1]]
        xe = np.zeros((C, D), dtype=np.float32)
        xe[: ids.size] = x[ids]
        # pack per-core images (see _build_bass docstring)
        xTr = xe.reshape(C, KD, 128).transpose(2, 1, 0).reshape(128, KD * C)
        HX = KD // 2
        grps = _w1_groups(KF)
        w1e = W1[e].reshape(KD, 128, KF, 128)
        parts = []
        f0 = 0
        for gw in grps:
            blk = w1e[:, :, f0 : f0 + gw]  # [KD, 128, gw, 128]
            parts.append(blk.transpose(1, 0, 2, 3).reshape(128, KD * gw * 128))
            f0 += gw
        w1r = np.concatenate([p.reshape(-1) for p in parts])  # flat group blocks
        w2r = (
            W2[e]
            .reshape(NP, W2P, 128, D)
            .transpose(0, 2, 1, 3)
            .reshape(NP, 128, W2P * D)
        )
        in_maps.append(
            {
                "xA": np.ascontiguousarray(xTr[:, : HX * C]).astype(np_io, copy=False),
                "xB": np.ascontiguousarray(xTr[:, HX * C :]).astype(np_io, copy=False),
                "w1": np.ascontiguousarray(w1r).astype(np_io, copy=False),
                "w2": np.ascontiguousarray(w2r).astype(np_io, copy=False),
                "b1r": np.ascontiguousarray(b1[e].reshape(KF, 128).T),
            }
        )

    res = run_bass_kernel_spmd(nc, in_maps, list(range(N_CORES)))

    out = np.zeros_like(x)
    for e in range(E):
        ids = order[starts[e] : starts[e + 1]]
        if ids.size == 0:
            continue
        yr = np.concatenate(
            [
                np.asarray(res.results[e]["yA"], dtype=np.float32),
                np.asarray(res.results[e]["yB"], dtype=np.float32),
            ],
            axis=1,
        )  # [128, KD*C]
        y = yr.reshape(128, KD, C).transpose(2, 1, 0).reshape(C, D)[: ids.size]
        out[ids] = gate[ids, None] * (y + b2[e])
    return out.reshape(orig_shape)
